# revision 3
# baseline (speedup 1.0000x reference)
"""CPGNN (compatibility-guided GNN) kernel for 8 Trainium2 NeuronCores.

Reference computation (N=10000, F=512, HID=256, C=16, 4 post iterations):
    h      = relu(normed_adj @ (features @ W1) + b1)
    logits = normed_adj @ (h @ W2) + b2
    E_hat  = softmax(logits) - 1/C
    B_hat  = E_hat;  4x: B_hat = E_hat + raw_adj @ (B_hat @ H)
    out    = B_hat + 1/C

Sharding: rows of both adjacency matrices are sharded over the 8 cores
(1280 rows per core, last core padded).  The adjacency shards are
uploaded TRANSPOSED (K-major, [10240, 1280] bf16) so every on-device
matmul can use natural layouts.  The small per-core [rows, C] matrices
are all-gathered between phases via device collectives.  All big
matmuls run in bf16 with fp32 PSUM accumulation (verified: end-to-end
relative error ~1.3e-3, at the fp32 reordering noise floor of this
amplifying iteration).
"""

import numpy as np
import ml_dtypes

RANKS = 8
P = 128
NREAL = 10000
NK = 10240            # padded global row count (80 k-tiles)
ML = 1280             # local rows per core (10 m-tiles)
KT = NK // P          # 80
MT = ML // P          # 10
F = 512
FT = F // P           # 4
HID = 256
C = 16
NPOST = 4
NRES = 44             # raw-adj k-tiles kept resident in SBUF after 1st pass
NCH = 3               # free-dim chunks of ML: 512/512/256
CHUNKS = [(0, 512), (512, 1024), (1024, 1280)]

_CACHE = {}


def _build_and_compile():
    import concourse.mybir as mybir
    import concourse.tile as tile
    from concourse import bacc
    from concourse.masks import make_identity

    dt = mybir.dt
    f32 = dt.float32
    bf16 = dt.bfloat16
    AF = mybir.ActivationFunctionType

    nc = bacc.Bacc("TRN2", target_bir_lowering=False, debug=False,
                   num_devices=RANKS)

    adjTn = nc.dram_tensor("adjTn", [NK, ML], bf16, kind="ExternalInput").ap()
    adjTr = nc.dram_tensor("adjTr", [NK, ML], bf16, kind="ExternalInput").ap()
    xT = nc.dram_tensor("xT", [F, NK], bf16, kind="ExternalInput").ap()
    w1 = nc.dram_tensor("w1", [F, HID], bf16, kind="ExternalInput").ap()
    w2 = nc.dram_tensor("w2", [HID, C], bf16, kind="ExternalInput").ap()
    hm = nc.dram_tensor("hm", [C, C], bf16, kind="ExternalInput").ap()
    b1 = nc.dram_tensor("b1", [HID, 1], f32, kind="ExternalInput").ap()
    b2r = nc.dram_tensor("b2r", [P, C], f32, kind="ExternalInput").ap()
    outT = nc.dram_tensor("outT", [C, ML], f32, kind="ExternalOutput").ap()

    rg = [list(range(RANKS))]

    with tile.TileContext(nc) as tc:
        with tc.tile_pool(name="const", bufs=1) as const_pool, \
             tc.tile_pool(name="persist", bufs=1) as persist, \
             tc.tile_pool(name="dram", bufs=1, space="DRAM") as dram_pool:

            # ---- constants ----
            w1_sb = const_pool.tile([P, FT, HID], bf16)
            nc.sync.dma_start(w1_sb[:], w1.rearrange("(kt p) h -> p kt h", p=P))
            w2_sb = const_pool.tile([P, 2, C], bf16)
            nc.sync.dma_start(w2_sb[:], w2.rearrange("(kt p) c -> p kt c", p=P))
            h_sb = const_pool.tile([C, C], bf16)
            nc.sync.dma_start(h_sb[:], hm[:])
            b1_sb = const_pool.tile([P, 2, 1], f32)
            nc.sync.dma_start(b1_sb[:], b1.rearrange("(t p) o -> p t o", p=P))
            b2r_sb = const_pool.tile([P, C], f32)
            nc.sync.dma_start(b2r_sb[:], b2r[:])
            ident = const_pool.tile([P, P], f32)
            make_identity(nc, ident)

            # ---- persistent intermediates ----
            h1t_sb = persist.tile([P, 2, ML], bf16)        # h.T  [HID, ML]
            hw2f_sb = persist.tile([P, KT, C], bf16)       # gathered h@W2 [NK, C]
            et_sb = persist.tile([C, ML], f32)             # E_hat.T local
            btcat_sb = persist.tile([C, NK], bf16)         # gathered B.T
            y_sb = persist.tile([P, KT, C], bf16)          # (B @ H) K-major
            e_sb = persist.tile([P, MT * C], f32)          # E_hat local natural

            # ================= phase 1: XW1 = X @ W1  [NK, HID] =============
            with tc.tile_pool(name="xw1p", bufs=1) as xw1p:
                xw1_sb = xw1p.tile([P, KT, HID], bf16)
                with tc.tile_pool(name="ph1", bufs=1) as ph1, \
                     tc.tile_pool(name="ps1", bufs=4, space="PSUM") as ps1:
                    xT_sb = ph1.tile([P, FT, NK], bf16)
                    nc.sync.dma_start(xT_sb[:],
                                      xT.rearrange("(kt p) n -> p kt n", p=P))
                    for m in range(KT):
                        psum1 = ps1.tile([P, HID], f32, name="psum1")
                        for kf in range(FT):
                            nc.tensor.matmul(
                                psum1[:],
                                xT_sb[:, kf, m * P:(m + 1) * P],
                                w1_sb[:, kf, :],
                                start=(kf == 0), stop=(kf == FT - 1))
                        nc.scalar.activation(xw1_sb[:, m, :], psum1[:], AF.Copy)

                # ============= phase 2: H1T = relu(XW1.T @ adjTn + b1) ======
                with tc.tile_pool(name="ph2s", bufs=4) as ph2s, \
                     tc.tile_pool(name="ps2", bufs=1, space="PSUM") as ps2:
                    psum_h0 = ps2.tile([P, ML], f32, name="psum_h0")
                    psum_h1 = ps2.tile([P, ML], f32, name="psum_h1")
                    psum_h = [psum_h0, psum_h1]
                    for k in range(KT):
                        adjn_k = ph2s.tile([P, ML], bf16, name="adjn_k")
                        nc.sync.dma_start(adjn_k[:], adjTn[k * P:(k + 1) * P, :])
                        for mh in range(2):
                            for (n0, n1) in CHUNKS:
                                nc.tensor.matmul(
                                    psum_h[mh][:, n0:n1],
                                    xw1_sb[:, k, mh * P:(mh + 1) * P],
                                    adjn_k[:, n0:n1],
                                    start=(k == 0), stop=(k == KT - 1))
                    for mh in range(2):
                        nc.scalar.activation(h1t_sb[:, mh, :], psum_h[mh][:],
                                             AF.Relu, bias=b1_sb[:, mh, :])

            # ================= phase 3: hW2 = h @ W2  [ML, C], all-gather ===
            with tc.tile_pool(name="ph3", bufs=1) as ph3, \
                 tc.tile_pool(name="ps3", bufs=4, space="PSUM") as ps3:
                hw2_sb = ph3.tile([P, MT, C], bf16)
                for m in range(MT):
                    psum3 = ps3.tile([P, C], f32, name="psum3")
                    for kh in range(2):
                        nc.tensor.matmul(
                            psum3[:],
                            h1t_sb[:, kh, m * P:(m + 1) * P],
                            w2_sb[:, kh, :],
                            start=(kh == 0), stop=(kh == 1))
                    nc.scalar.activation(hw2_sb[:, m, :], psum3[:], AF.Copy)
                hw2loc_dram = dram_pool.tile([ML, C], bf16)
                nc.sync.dma_start(
                    hw2loc_dram.rearrange("(mt p) c -> p mt c", p=P), hw2_sb[:])
                hw2full_dram = dram_pool.tile([NK, C], bf16, addr_space="Shared")
                nc.gpsimd.collective_compute(
                    "AllGather", mybir.AluOpType.bypass, replica_groups=rg,
                    ins=[hw2loc_dram[:].opt()], outs=[hw2full_dram[:].opt()])
                nc.sync.dma_start(
                    hw2f_sb[:], hw2full_dram.rearrange("(kt p) c -> p kt c", p=P))

            # ====== phase 4: logitsT = hW2_full.T @ adjTn; softmax; E_hat ===
            with tc.tile_pool(name="ph4s", bufs=4) as ph4s, \
                 tc.tile_pool(name="ph4", bufs=1) as ph4, \
                 tc.tile_pool(name="ps4", bufs=1, space="PSUM") as ps4, \
                 tc.tile_pool(name="ps4t", bufs=2, space="PSUM") as ps4t:
                psum_l = ps4.tile([C, ML], f32, name="psum_l")
                for k in range(KT):
                    adjn_k2 = ph4s.tile([P, ML], bf16, name="adjn_k2")
                    nc.sync.dma_start(adjn_k2[:], adjTn[k * P:(k + 1) * P, :])
                    for (n0, n1) in CHUNKS:
                        nc.tensor.matmul(
                            psum_l[:, n0:n1],
                            hw2f_sb[:, k, :],
                            adjn_k2[:, n0:n1],
                            start=(k == 0), stop=(k == KT - 1))
                lt_sb = ph4.tile([C, ML], f32)
                nc.scalar.activation(lt_sb[:], psum_l[:], AF.Copy)

                etb_sb = ph4.tile([C, ML], bf16)
                for m in range(MT):
                    # logits m-tile, natural layout: transpose [C,P] -> [P,C]
                    tp_ps = ps4t.tile([P, C], f32, name="tp_ps")
                    nc.tensor.transpose(tp_ps[:], lt_sb[:, m * P:(m + 1) * P],
                                        ident[:C, :C])
                    lg_m = ph4.tile([P, C], f32, name="lg_m", bufs=2)
                    nc.vector.tensor_add(lg_m[:], tp_ps[:], b2r_sb[:])
                    # softmax over free axis
                    mx = ph4.tile([P, 1], f32, name="mx", bufs=2)
                    nc.vector.reduce_max(mx[:], lg_m[:], axis=mybir.AxisListType.X)
                    negmx = ph4.tile([P, 1], f32, name="negmx", bufs=2)
                    nc.vector.tensor_scalar_mul(negmx[:], mx[:], -1.0)
                    ex = ph4.tile([P, C], f32, name="ex", bufs=2)
                    nc.scalar.activation(ex[:], lg_m[:], AF.Exp, bias=negmx[:])
                    sm = ph4.tile([P, 1], f32, name="sm", bufs=2)
                    nc.vector.reduce_sum(sm[:], ex[:], axis=mybir.AxisListType.X)
                    rs = ph4.tile([P, 1], f32, name="rs", bufs=2)
                    nc.vector.reciprocal(rs[:], sm[:])
                    # E_hat = ex * rs - 1/C
                    nc.vector.tensor_scalar(
                        e_sb[:, m * C:(m + 1) * C], ex[:], rs[:], -1.0 / C,
                        op0=mybir.AluOpType.mult, op1=mybir.AluOpType.add)
                    # E_hat.T block
                    tp2_ps = ps4t.tile([C, P], f32, name="tp2_ps")
                    nc.tensor.transpose(tp2_ps[:], e_sb[:, m * C:(m + 1) * C],
                                        ident[:])
                    nc.scalar.activation(et_sb[:, m * P:(m + 1) * P], tp2_ps[:],
                                         AF.Copy)
                    nc.scalar.activation(etb_sb[:, m * P:(m + 1) * P], tp2_ps[:],
                                         AF.Copy)

                # all-gather E_hat.T blocks -> btcat
                et_dram = dram_pool.tile([C, ML], bf16)
                nc.sync.dma_start(et_dram[:], etb_sb[:])
                btfull0 = dram_pool.tile([P, ML], bf16, addr_space="Shared")
                nc.gpsimd.collective_compute(
                    "AllGather", mybir.AluOpType.bypass, replica_groups=rg,
                    ins=[et_dram[:].opt()], outs=[btfull0[:].opt()])
                for r in range(RANKS):
                    nc.sync.dma_start(btcat_sb[:, r * ML:(r + 1) * ML],
                                      btfull0[C * r:C * (r + 1), :])

            # ================= phase 5: post-process iterations =============
            with tc.tile_pool(name="res", bufs=1) as res_pool, \
                 tc.tile_pool(name="ph5s", bufs=4) as ph5s, \
                 tc.tile_pool(name="ph5", bufs=1) as ph5, \
                 tc.tile_pool(name="ps5y", bufs=4, space="PSUM") as ps5y, \
                 tc.tile_pool(name="ps5b", bufs=1, space="PSUM") as ps5b:
                adjr_res = res_pool.tile([P, NRES, ML], bf16)
                for it in range(NPOST):
                    # Y = B @ H in K-major layout, from gathered B.T blocks
                    for m in range(KT):
                        psum_y = ps5y.tile([P, C], f32, name="psum_y")
                        nc.tensor.matmul(psum_y[:],
                                         btcat_sb[:, m * P:(m + 1) * P],
                                         h_sb[:], start=True, stop=True)
                        nc.scalar.activation(y_sb[:, m, :], psum_y[:], AF.Copy)
                    # T.T = Y.T @ adjTr  (accumulate over k-tiles)
                    psum_b = ps5b.tile([C, ML], f32, name="psum_b")
                    for k in range(KT):
                        if k < NRES:
                            if it == 0:
                                nc.sync.dma_start(adjr_res[:, k, :],
                                                  adjTr[k * P:(k + 1) * P, :])
                            src = adjr_res[:, k, :]
                        else:
                            adjr_k = ph5s.tile([P, ML], bf16, name="adjr_k")
                            nc.sync.dma_start(adjr_k[:],
                                              adjTr[k * P:(k + 1) * P, :])
                            src = adjr_k[:]
                        for (n0, n1) in CHUNKS:
                            nc.tensor.matmul(
                                psum_b[:, n0:n1],
                                y_sb[:, k, :],
                                src[:, n0:n1],
                                start=(k == 0), stop=(k == KT - 1))
                    if it < NPOST - 1:
                        btnb = ph5.tile([C, ML], bf16, name="btnb", bufs=2)
                        nc.vector.tensor_add(btnb[:], psum_b[:], et_sb[:])
                        bt_dram = dram_pool.tile([C, ML], bf16,
                                                 name=f"bt_dram{it}")
                        nc.sync.dma_start(bt_dram[:], btnb[:])
                        btfull = dram_pool.tile([P, ML], bf16,
                                                name=f"btfull{it}",
                                                addr_space="Shared")
                        nc.gpsimd.collective_compute(
                            "AllGather", mybir.AluOpType.bypass,
                            replica_groups=rg,
                            ins=[bt_dram[:].opt()], outs=[btfull[:].opt()])
                        for r in range(RANKS):
                            nc.sync.dma_start(
                                btcat_sb[:, r * ML:(r + 1) * ML],
                                btfull[C * r:C * (r + 1), :])
                    else:
                        btn = ph5.tile([C, ML], f32, name="btn")
                        nc.vector.tensor_add(btn[:], psum_b[:], et_sb[:])
                        outT_sb = ph5.tile([C, ML], f32, name="outT_sb")
                        nc.vector.tensor_scalar_add(outT_sb[:], btn[:], 1.0 / C)
                        nc.sync.dma_start(outT[:], outT_sb[:])

    nc.compile()
    return nc


def _get_compiled():
    if "nc" not in _CACHE:
        _CACHE["nc"] = _build_and_compile()
    return _CACHE["nc"]


def _prep_inputs(raw_adj, normed_adj, features, W1, b1, W2, b2, H):
    bf = ml_dtypes.bfloat16
    xTp = np.zeros((F, NK), dtype=bf)
    xTp[:, :NREAL] = np.ascontiguousarray(features.T).astype(bf)
    w1b = np.ascontiguousarray(W1).astype(bf)
    w2b = np.ascontiguousarray(W2).astype(bf)
    hb = np.ascontiguousarray(H).astype(bf)
    b1c = np.asarray(b1, dtype=np.float32).reshape(HID, 1).copy()
    b2rep = np.broadcast_to(np.asarray(b2, dtype=np.float32), (P, C)).copy()
    in_maps = []
    for r in range(RANKS):
        r0 = r * ML
        r1 = min(r0 + ML, NREAL)
        nr = r1 - r0
        an = np.zeros((NK, ML), dtype=bf)
        an[:NREAL, :nr] = np.ascontiguousarray(normed_adj[r0:r1].T).astype(bf)
        ar = np.zeros((NK, ML), dtype=bf)
        ar[:NREAL, :nr] = np.ascontiguousarray(raw_adj[r0:r1].T).astype(bf)
        in_maps.append({
            "adjTn": an, "adjTr": ar, "xT": xTp, "w1": w1b, "w2": w2b,
            "hm": hb, "b1": b1c, "b2r": b2rep,
        })
    return in_maps


def run_on_device(in_maps, trace=False):
    from concourse import bass_utils
    nc = _get_compiled()
    return bass_utils.run_bass_kernel_spmd(
        nc, in_maps, core_ids=list(range(RANKS)), trace=trace)


def kernel(raw_adj, normed_adj, features, y_onehot, train_mask,
           W1, b1, W2, b2, H):
    in_maps = _prep_inputs(np.asarray(raw_adj), np.asarray(normed_adj),
                           np.asarray(features), np.asarray(W1),
                           np.asarray(b1), np.asarray(W2), np.asarray(b2),
                           np.asarray(H))
    res = run_on_device(in_maps)
    parts = []
    for r in range(RANKS):
        o = np.asarray(res.results[r]["outT"], dtype=np.float32)  # [C, ML]
        parts.append(o.T)
    full = np.concatenate(parts, axis=0)[:NREAL]
    return np.ascontiguousarray(full).astype(np.float32)


# revision 7
# speedup vs baseline: 1435.5312x; 1435.5312x over previous
"""CPGNN (compatibility-guided GNN) kernel for 8 Trainium2 NeuronCores.

Reference computation (N=10000, F=512, HID=256, C=16, 4 post iterations):
    h      = relu(normed_adj @ (features @ W1) + b1)
    logits = normed_adj @ (h @ W2) + b2
    E_hat  = softmax(logits) - 1/C
    B_hat  = E_hat;  4x: B_hat = E_hat + raw_adj @ (B_hat @ H)
    out    = B_hat + 1/C

Sharding: rows of both adjacency matrices are sharded over the 8 cores
(1280 rows per core, last core padded).  The adjacency shards are
uploaded TRANSPOSED (K-major, [10240, 1280] bf16) so every on-device
matmul can use natural layouts.  The small per-core [rows, C] matrices
are all-gathered between phases via device collectives.  All big
matmuls run in bf16 with fp32 PSUM accumulation (verified: end-to-end
relative error ~1.3e-3, at the fp32 reordering noise floor of this
amplifying iteration).
"""

import numpy as np
import ml_dtypes

RANKS = 8
P = 128
NREAL = 10000
NK = 10240            # padded global row count (80 k-tiles)
ML = 1280             # local rows per core (10 m-tiles)
KT = NK // P          # 80
MT = ML // P          # 10
F = 512
FT = F // P           # 4
HID = 256
C = 16
NPOST = 4
NRES = 44             # raw-adj k-tiles kept resident in SBUF after 1st pass
NRESN = 36            # normed-adj k-tiles cached in SBUF between ph2 and ph4
NCH = 3               # free-dim chunks of ML: 512/512/256
CHUNKS = [(0, 512), (512, 1024), (1024, 1280)]

_CACHE = {}


def _build_and_compile():
    import concourse.mybir as mybir
    import concourse.tile as tile
    from concourse import bacc
    from concourse.masks import make_identity

    dt = mybir.dt
    f32 = dt.float32
    bf16 = dt.bfloat16
    AF = mybir.ActivationFunctionType

    nc = bacc.Bacc("TRN2", target_bir_lowering=False, debug=False,
                   num_devices=RANKS)

    adjTn = nc.dram_tensor("adjTn", [NK, ML], bf16, kind="ExternalInput").ap()
    adjTr = nc.dram_tensor("adjTr", [NK, ML], bf16, kind="ExternalInput").ap()
    xT = nc.dram_tensor("xT", [F, NK], bf16, kind="ExternalInput").ap()
    w1 = nc.dram_tensor("w1", [F, HID], bf16, kind="ExternalInput").ap()
    w2 = nc.dram_tensor("w2", [HID, C], bf16, kind="ExternalInput").ap()
    hm = nc.dram_tensor("hm", [C, C], bf16, kind="ExternalInput").ap()
    b1 = nc.dram_tensor("b1", [HID, 1], f32, kind="ExternalInput").ap()
    b2r = nc.dram_tensor("b2r", [P, C], f32, kind="ExternalInput").ap()
    outT = nc.dram_tensor("outT", [C, ML], f32, kind="ExternalOutput").ap()

    rg = [list(range(RANKS))]

    with tile.TileContext(nc) as tc:
        with tc.tile_pool(name="const", bufs=1) as const_pool, \
             tc.tile_pool(name="persist", bufs=1) as persist, \
             tc.tile_pool(name="dram", bufs=1, space="DRAM") as dram_pool:

            # ---- constants ----
            w1_sb = const_pool.tile([P, FT, HID], bf16)
            nc.sync.dma_start(w1_sb[:], w1.rearrange("(kt p) h -> p kt h", p=P))
            w2_sb = const_pool.tile([P, 2, C], bf16)
            nc.sync.dma_start(w2_sb[:], w2.rearrange("(kt p) c -> p kt c", p=P))
            h_sb = const_pool.tile([C, C], bf16)
            nc.sync.dma_start(h_sb[:], hm[:])
            b1_sb = const_pool.tile([P, 2, 1], f32)
            nc.sync.dma_start(b1_sb[:], b1.rearrange("(t p) o -> p t o", p=P))
            b2r_sb = const_pool.tile([P, C], f32)
            nc.sync.dma_start(b2r_sb[:], b2r[:])
            ident = const_pool.tile([P, P], f32)
            make_identity(nc, ident)

            # ---- persistent intermediates ----
            h1t_sb = persist.tile([P, 2, ML], bf16)        # h.T  [HID, ML]
            hw2f_sb = persist.tile([P, KT, C], bf16)       # gathered h@W2 [NK, C]
            et_sb = persist.tile([C, ML], f32)             # E_hat.T local
            btcat_sb = persist.tile([C, NK], bf16)         # gathered B.T
            y_sb = persist.tile([P, KT, C], bf16)          # (B @ H) K-major
            e_sb = persist.tile([P, MT * C], f32)          # E_hat local natural

            # ================= phase 1: XW1 = X @ W1  [NK, HID] =============
            with tc.tile_pool(name="xw1p", bufs=1) as xw1p:
                xw1_sb = xw1p.tile([P, KT, HID], bf16)
                with tc.tile_pool(name="ph1", bufs=1) as ph1, \
                     tc.tile_pool(name="ps1", bufs=4, space="PSUM") as ps1:
                    xT_sb = ph1.tile([P, FT, NK], bf16)
                    xT_r = xT.rearrange("(kt p) n -> p kt n", p=P)
                    XCH = 8
                    xw = NK // XCH
                    for c in range(XCH):
                        nc.sync.dma_start(xT_sb[:, :, c * xw:(c + 1) * xw],
                                          xT_r[:, :, c * xw:(c + 1) * xw])
                    for m in range(KT):
                        psum1 = ps1.tile([P, HID], f32, name="psum1")
                        for kf in range(FT):
                            nc.tensor.matmul(
                                psum1[:],
                                xT_sb[:, kf, m * P:(m + 1) * P],
                                w1_sb[:, kf, :],
                                start=(kf == 0), stop=(kf == FT - 1))
                        nc.scalar.activation(xw1_sb[:, m, :], psum1[:], AF.Copy)

                # ============= phase 2: H1T = relu(XW1.T @ adjTn + b1) ======
                # cachen outlives phase 2 (reused in phase 4)
                cachen = tc.tile_pool(name="cachen", bufs=1)
                cachen.__enter__()
                adjn_res = cachen.tile([P, NRESN, ML], bf16)
                with tc.tile_pool(name="ph2s", bufs=4) as ph2s, \
                     tc.tile_pool(name="ps2", bufs=1, space="PSUM") as ps2:
                    psum_h0 = ps2.tile([P, ML], f32, name="psum_h0")
                    psum_h1 = ps2.tile([P, ML], f32, name="psum_h1")
                    psum_h = [psum_h0, psum_h1]
                    for k in range(KT):
                        if k < NRESN:
                            nc.sync.dma_start(adjn_res[:, k, :],
                                              adjTn[k * P:(k + 1) * P, :])
                            src = adjn_res[:, k, :]
                        else:
                            adjn_k = ph2s.tile([P, ML], bf16, name="adjn_k")
                            nc.sync.dma_start(adjn_k[:],
                                              adjTn[k * P:(k + 1) * P, :])
                            src = adjn_k[:]
                        for mh in range(2):
                            for (n0, n1) in CHUNKS:
                                nc.tensor.matmul(
                                    psum_h[mh][:, n0:n1],
                                    xw1_sb[:, k, mh * P:(mh + 1) * P],
                                    src[:, n0:n1],
                                    start=(k == 0), stop=(k == KT - 1))
                    for mh in range(2):
                        nc.scalar.activation(h1t_sb[:, mh, :], psum_h[mh][:],
                                             AF.Relu, bias=b1_sb[:, mh, :])

            # ================= phase 3: hW2 = h @ W2  [ML, C], all-gather ===
            with tc.tile_pool(name="ph3", bufs=1) as ph3, \
                 tc.tile_pool(name="ps3", bufs=4, space="PSUM") as ps3:
                hw2_sb = ph3.tile([P, MT, C], bf16)
                for m in range(MT):
                    psum3 = ps3.tile([P, C], f32, name="psum3")
                    for kh in range(2):
                        nc.tensor.matmul(
                            psum3[:],
                            h1t_sb[:, kh, m * P:(m + 1) * P],
                            w2_sb[:, kh, :],
                            start=(kh == 0), stop=(kh == 1))
                    nc.scalar.activation(hw2_sb[:, m, :], psum3[:], AF.Copy)
                hw2loc_dram = dram_pool.tile([ML, C], bf16)
                nc.sync.dma_start(
                    hw2loc_dram.rearrange("(mt p) c -> p mt c", p=P), hw2_sb[:])
                hw2full_dram = dram_pool.tile([NK, C], bf16, addr_space="Shared")
                nc.gpsimd.collective_compute(
                    "AllGather", mybir.AluOpType.bypass, replica_groups=rg,
                    ins=[hw2loc_dram[:].opt()], outs=[hw2full_dram[:].opt()])
                nc.sync.dma_start(
                    hw2f_sb[:], hw2full_dram.rearrange("(kt p) c -> p kt c", p=P))

            # ====== phase 4: logitsT = hW2_full.T @ adjTn; softmax; E_hat ===
            with tc.tile_pool(name="ph4s", bufs=4) as ph4s, \
                 tc.tile_pool(name="ph4", bufs=1) as ph4, \
                 tc.tile_pool(name="ps4", bufs=1, space="PSUM") as ps4, \
                 tc.tile_pool(name="ps4t", bufs=2, space="PSUM") as ps4t:
                psum_l = ps4.tile([C, ML], f32, name="psum_l")
                for k in range(KT):
                    if k < NRESN:
                        src = adjn_res[:, k, :]
                    else:
                        adjn_k2 = ph4s.tile([P, ML], bf16, name="adjn_k2")
                        nc.sync.dma_start(adjn_k2[:], adjTn[k * P:(k + 1) * P, :])
                        src = adjn_k2[:]
                    for (n0, n1) in CHUNKS:
                        nc.tensor.matmul(
                            psum_l[:, n0:n1],
                            hw2f_sb[:, k, :],
                            src[:, n0:n1],
                            start=(k == 0), stop=(k == KT - 1))
                lt_sb = ph4.tile([C, ML], f32)
                nc.scalar.activation(lt_sb[:], psum_l[:], AF.Copy)

                etb_sb = ph4.tile([C, ML], bf16)
                for m in range(MT):
                    # logits m-tile, natural layout: transpose [C,P] -> [P,C]
                    tp_ps = ps4t.tile([P, C], f32, name="tp_ps")
                    nc.tensor.transpose(tp_ps[:], lt_sb[:, m * P:(m + 1) * P],
                                        ident[:C, :C])
                    lg_m = ph4.tile([P, C], f32, name="lg_m", bufs=2)
                    nc.vector.tensor_add(lg_m[:], tp_ps[:], b2r_sb[:])
                    # softmax over free axis
                    mx = ph4.tile([P, 1], f32, name="mx", bufs=2)
                    nc.vector.reduce_max(mx[:], lg_m[:], axis=mybir.AxisListType.X)
                    negmx = ph4.tile([P, 1], f32, name="negmx", bufs=2)
                    nc.vector.tensor_scalar_mul(negmx[:], mx[:], -1.0)
                    ex = ph4.tile([P, C], f32, name="ex", bufs=2)
                    nc.scalar.activation(ex[:], lg_m[:], AF.Exp, bias=negmx[:])
                    sm = ph4.tile([P, 1], f32, name="sm", bufs=2)
                    nc.vector.reduce_sum(sm[:], ex[:], axis=mybir.AxisListType.X)
                    rs = ph4.tile([P, 1], f32, name="rs", bufs=2)
                    nc.vector.reciprocal(rs[:], sm[:])
                    # E_hat = ex * rs - 1/C
                    nc.vector.tensor_scalar(
                        e_sb[:, m * C:(m + 1) * C], ex[:], rs[:], -1.0 / C,
                        op0=mybir.AluOpType.mult, op1=mybir.AluOpType.add)
                    # E_hat.T block
                    tp2_ps = ps4t.tile([C, P], f32, name="tp2_ps")
                    nc.tensor.transpose(tp2_ps[:], e_sb[:, m * C:(m + 1) * C],
                                        ident[:])
                    nc.scalar.activation(et_sb[:, m * P:(m + 1) * P], tp2_ps[:],
                                         AF.Copy)
                    nc.scalar.activation(etb_sb[:, m * P:(m + 1) * P], tp2_ps[:],
                                         AF.Copy)

                # all-gather E_hat.T blocks -> btcat
                et_dram = dram_pool.tile([C, ML], bf16)
                nc.sync.dma_start(et_dram[:], etb_sb[:])
                btfull0 = dram_pool.tile([P, ML], bf16, addr_space="Shared")
                nc.gpsimd.collective_compute(
                    "AllGather", mybir.AluOpType.bypass, replica_groups=rg,
                    ins=[et_dram[:].opt()], outs=[btfull0[:].opt()])
                for r in range(RANKS):
                    nc.sync.dma_start(btcat_sb[:, r * ML:(r + 1) * ML],
                                      btfull0[C * r:C * (r + 1), :])

            cachen.__exit__(None, None, None)

            # ================= phase 5: post-process iterations =============
            with tc.tile_pool(name="res", bufs=1) as res_pool, \
                 tc.tile_pool(name="ph5s", bufs=4) as ph5s, \
                 tc.tile_pool(name="ph5", bufs=1) as ph5, \
                 tc.tile_pool(name="ps5y", bufs=4, space="PSUM") as ps5y, \
                 tc.tile_pool(name="ps5b", bufs=1, space="PSUM") as ps5b:
                adjr_res = res_pool.tile([P, NRES, ML], bf16)
                for it in range(NPOST):
                    # Y = B @ H in K-major layout, from gathered B.T blocks
                    for m in range(KT):
                        psum_y = ps5y.tile([P, C], f32, name="psum_y")
                        nc.tensor.matmul(psum_y[:],
                                         btcat_sb[:, m * P:(m + 1) * P],
                                         h_sb[:], start=True, stop=True)
                        nc.scalar.activation(y_sb[:, m, :], psum_y[:], AF.Copy)
                    # T.T = Y.T @ adjTr  (accumulate over k-tiles)
                    psum_b = ps5b.tile([C, ML], f32, name="psum_b")
                    for k in range(KT):
                        if k < NRES:
                            if it == 0:
                                nc.sync.dma_start(adjr_res[:, k, :],
                                                  adjTr[k * P:(k + 1) * P, :])
                            src = adjr_res[:, k, :]
                        else:
                            adjr_k = ph5s.tile([P, ML], bf16, name="adjr_k")
                            nc.sync.dma_start(adjr_k[:],
                                              adjTr[k * P:(k + 1) * P, :])
                            src = adjr_k[:]
                        for (n0, n1) in CHUNKS:
                            nc.tensor.matmul(
                                psum_b[:, n0:n1],
                                y_sb[:, k, :],
                                src[:, n0:n1],
                                start=(k == 0), stop=(k == KT - 1))
                    if it < NPOST - 1:
                        btnb = ph5.tile([C, ML], bf16, name="btnb", bufs=2)
                        nc.vector.tensor_add(btnb[:], psum_b[:], et_sb[:])
                        bt_dram = dram_pool.tile([C, ML], bf16,
                                                 name=f"bt_dram{it}")
                        nc.sync.dma_start(bt_dram[:], btnb[:])
                        btfull = dram_pool.tile([P, ML], bf16,
                                                name=f"btfull{it}",
                                                addr_space="Shared")
                        nc.gpsimd.collective_compute(
                            "AllGather", mybir.AluOpType.bypass,
                            replica_groups=rg,
                            ins=[bt_dram[:].opt()], outs=[btfull[:].opt()])
                        for r in range(RANKS):
                            nc.sync.dma_start(
                                btcat_sb[:, r * ML:(r + 1) * ML],
                                btfull[C * r:C * (r + 1), :])
                    else:
                        btn = ph5.tile([C, ML], f32, name="btn")
                        nc.vector.tensor_add(btn[:], psum_b[:], et_sb[:])
                        outT_sb = ph5.tile([C, ML], f32, name="outT_sb")
                        nc.vector.tensor_scalar_add(outT_sb[:], btn[:], 1.0 / C)
                        nc.sync.dma_start(outT[:], outT_sb[:])

    nc.compile()
    return nc


def _get_compiled():
    if "nc" not in _CACHE:
        _CACHE["nc"] = _build_and_compile()
    return _CACHE["nc"]


def _prep_inputs(raw_adj, normed_adj, features, W1, b1, W2, b2, H):
    bf = ml_dtypes.bfloat16
    xTp = np.zeros((F, NK), dtype=bf)
    xTp[:, :NREAL] = np.ascontiguousarray(features.T).astype(bf)
    w1b = np.ascontiguousarray(W1).astype(bf)
    w2b = np.ascontiguousarray(W2).astype(bf)
    hb = np.ascontiguousarray(H).astype(bf)
    b1c = np.asarray(b1, dtype=np.float32).reshape(HID, 1).copy()
    b2rep = np.broadcast_to(np.asarray(b2, dtype=np.float32), (P, C)).copy()
    in_maps = []
    for r in range(RANKS):
        r0 = r * ML
        r1 = min(r0 + ML, NREAL)
        nr = r1 - r0
        an = np.zeros((NK, ML), dtype=bf)
        an[:NREAL, :nr] = np.ascontiguousarray(normed_adj[r0:r1].T).astype(bf)
        ar = np.zeros((NK, ML), dtype=bf)
        ar[:NREAL, :nr] = np.ascontiguousarray(raw_adj[r0:r1].T).astype(bf)
        in_maps.append({
            "adjTn": an, "adjTr": ar, "xT": xTp, "w1": w1b, "w2": w2b,
            "hm": hb, "b1": b1c, "b2r": b2rep,
        })
    return in_maps


def run_on_device(in_maps, trace=False):
    from concourse import bass_utils
    nc = _get_compiled()
    return bass_utils.run_bass_kernel_spmd(
        nc, in_maps, core_ids=list(range(RANKS)), trace=trace)


def kernel(raw_adj, normed_adj, features, y_onehot, train_mask,
           W1, b1, W2, b2, H):
    in_maps = _prep_inputs(np.asarray(raw_adj), np.asarray(normed_adj),
                           np.asarray(features), np.asarray(W1),
                           np.asarray(b1), np.asarray(W2), np.asarray(b2),
                           np.asarray(H))
    res = run_on_device(in_maps)
    parts = []
    for r in range(RANKS):
        o = np.asarray(res.results[r]["outT"], dtype=np.float32)  # [C, ML]
        parts.append(o.T)
    full = np.concatenate(parts, axis=0)[:NREAL]
    return np.ascontiguousarray(full).astype(np.float32)


# revision 9
# speedup vs baseline: 24654.5031x; 17.1745x over previous
"""CPGNN (compatibility-guided GNN) kernel for 8 Trainium2 NeuronCores.

Reference computation (N=10000, F=512, HID=256, C=16, 4 post iterations):
    h      = relu(normed_adj @ (features @ W1) + b1)
    logits = normed_adj @ (h @ W2) + b2
    E_hat  = softmax(logits) - 1/C
    B_hat  = E_hat;  4x: B_hat = E_hat + raw_adj @ (B_hat @ H)
    out    = B_hat + 1/C

Sharding: rows of both adjacency matrices are sharded over the 8 cores
(1280 rows per core, tail core padded).  The adjacency shards are
uploaded TRANSPOSED (K-major, [10240, 1280] bf16) so every on-device
matmul can use natural layouts.  The small per-core [rows, C] matrices
are all-gathered between phases via device collectives.  All big
matmuls run in bf16 with fp32 PSUM accumulation (verified: end-to-end
relative error ~3.7e-3, at the fp32 reordering noise floor of this
amplifying iteration).
"""

import os

import numpy as np
import ml_dtypes

RANKS = 8
P = 128
NREAL = 10000
NK = 10240            # padded global row count (80 k-tiles)
ML = 1280             # local rows per core (10 m-tiles)
KT = NK // P          # 80
MT = ML // P          # 10
F = 512
FT = F // P           # 4
HID = 256
C = 16
NPOST = 4
NRES = 44             # raw-adj k-tiles kept resident in SBUF after 1st pass
NRESN = 36            # normed-adj k-tiles cached in SBUF between ph2 and ph4
NCH = 3               # free-dim chunks of ML: 512/512/256
CHUNKS = [(0, 512), (512, 1024), (1024, 1280)]

PHASES = int(os.environ.get("CPGNN_PHASES", "5"))

_CACHE = {}


def _build_and_compile():
    import concourse.mybir as mybir
    import concourse.tile as tile
    from concourse import bacc
    from concourse.masks import make_identity

    dt = mybir.dt
    f32 = dt.float32
    bf16 = dt.bfloat16
    AF = mybir.ActivationFunctionType

    nc = bacc.Bacc("TRN2", target_bir_lowering=False, debug=False,
                   num_devices=RANKS)

    adjTn = nc.dram_tensor("adjTn", [NK, ML], bf16, kind="ExternalInput").ap()
    adjTr = nc.dram_tensor("adjTr", [NK, ML], bf16, kind="ExternalInput").ap()
    xT = nc.dram_tensor("xT", [F, NK], bf16, kind="ExternalInput").ap()
    w1 = nc.dram_tensor("w1", [F, HID], bf16, kind="ExternalInput").ap()
    w2 = nc.dram_tensor("w2", [HID, C], bf16, kind="ExternalInput").ap()
    hm = nc.dram_tensor("hm", [C, C], bf16, kind="ExternalInput").ap()
    b1 = nc.dram_tensor("b1", [HID, 1], f32, kind="ExternalInput").ap()
    b2r = nc.dram_tensor("b2r", [P, C], f32, kind="ExternalInput").ap()
    outT = nc.dram_tensor("outT", [C, ML], f32, kind="ExternalOutput").ap()

    rg = [list(range(RANKS))]

    with tile.TileContext(nc) as tc:
        with tc.tile_pool(name="const", bufs=1) as const_pool, \
             tc.tile_pool(name="persist", bufs=1) as persist, \
             tc.tile_pool(name="dram", bufs=1, space="DRAM") as dram_pool:

            # ---- constants ----
            w1_sb = const_pool.tile([P, FT, HID], bf16)
            nc.sync.dma_start(w1_sb[:], w1.rearrange("(kt p) h -> p kt h", p=P))
            w2_sb = const_pool.tile([P, 2, C], bf16)
            nc.sync.dma_start(w2_sb[:], w2.rearrange("(kt p) c -> p kt c", p=P))
            h_sb = const_pool.tile([C, C], bf16)
            nc.sync.dma_start(h_sb[:], hm[:])
            b1_sb = const_pool.tile([P, 2, 1], f32)
            nc.sync.dma_start(b1_sb[:], b1.rearrange("(t p) o -> p t o", p=P))
            b2r_sb = const_pool.tile([P, C], f32)
            nc.sync.dma_start(b2r_sb[:], b2r[:])
            ident = const_pool.tile([P, P], f32)
            make_identity(nc, ident)

            # ---- persistent intermediates ----
            h1t_sb = persist.tile([P, 2, ML], bf16)        # h.T  [HID, ML]
            hw2f_sb = persist.tile([P, KT, C], bf16)       # gathered h@W2 [NK, C]
            et_sb = persist.tile([C, ML], f32)             # E_hat.T local
            btcat_sb = persist.tile([C, NK], bf16)         # gathered B.T
            y_sb = persist.tile([P, KT, C], bf16)          # (B @ H) K-major
            e_sb = persist.tile([P, MT * C], f32)          # E_hat local natural

            # ================= phase 1: XW1 = X @ W1  [NK, HID] =============
            with tc.tile_pool(name="xw1p", bufs=1) as xw1p:
                xw1_sb = xw1p.tile([P, KT, HID], bf16)
                with tc.tile_pool(name="ph1", bufs=1) as ph1, \
                     tc.tile_pool(name="ps1", bufs=4, space="PSUM") as ps1:
                    xT_sb = ph1.tile([P, FT, NK], bf16)
                    xT_r = xT.rearrange("(kt p) n -> p kt n", p=P)
                    XCH = 8
                    xw = NK // XCH
                    for c in range(XCH):
                        nc.sync.dma_start(xT_sb[:, :, c * xw:(c + 1) * xw],
                                          xT_r[:, :, c * xw:(c + 1) * xw])
                    for m in range(KT):
                        psum1 = ps1.tile([P, HID], f32, name="psum1")
                        for kf in range(FT):
                            nc.tensor.matmul(
                                psum1[:],
                                xT_sb[:, kf, m * P:(m + 1) * P],
                                w1_sb[:, kf, :],
                                start=(kf == 0), stop=(kf == FT - 1))
                        nc.scalar.activation(xw1_sb[:, m, :], psum1[:], AF.Copy)

                # ============= phase 2: H1T = relu(XW1.T @ adjTn + b1) ======
                if PHASES >= 2:
                    # cachen outlives phase 2 (reused in phase 4)
                    cachen = tc.tile_pool(name="cachen", bufs=1)
                    cachen.__enter__()
                    adjn_res = cachen.tile([P, NRESN, ML], bf16)
                    with tc.tile_pool(name="ph2s", bufs=4) as ph2s, \
                         tc.tile_pool(name="ps2", bufs=1, space="PSUM") as ps2:
                        psum_h0 = ps2.tile([P, ML], f32, name="psum_h0")
                        psum_h1 = ps2.tile([P, ML], f32, name="psum_h1")
                        psum_h = [psum_h0, psum_h1]
                        for k in range(KT):
                            if k < NRESN:
                                nc.sync.dma_start(adjn_res[:, k, :],
                                                  adjTn[k * P:(k + 1) * P, :])
                                src = adjn_res[:, k, :]
                            else:
                                adjn_k = ph2s.tile([P, ML], bf16, name="adjn_k")
                                nc.sync.dma_start(adjn_k[:],
                                                  adjTn[k * P:(k + 1) * P, :])
                                src = adjn_k[:]
                            for mh in range(2):
                                for (n0, n1) in CHUNKS:
                                    nc.tensor.matmul(
                                        psum_h[mh][:, n0:n1],
                                        xw1_sb[:, k, mh * P:(mh + 1) * P],
                                        src[:, n0:n1],
                                        start=(k == 0), stop=(k == KT - 1))
                        for mh in range(2):
                            nc.scalar.activation(h1t_sb[:, mh, :], psum_h[mh][:],
                                                 AF.Relu, bias=b1_sb[:, mh, :])

            # ================= phase 3: hW2 = h @ W2  [ML, C], all-gather ===
            if PHASES >= 3:
                with tc.tile_pool(name="ph3", bufs=1) as ph3, \
                     tc.tile_pool(name="ps3", bufs=4, space="PSUM") as ps3:
                    hw2_sb = ph3.tile([P, MT, C], bf16)
                    for m in range(MT):
                        psum3 = ps3.tile([P, C], f32, name="psum3")
                        for kh in range(2):
                            nc.tensor.matmul(
                                psum3[:],
                                h1t_sb[:, kh, m * P:(m + 1) * P],
                                w2_sb[:, kh, :],
                                start=(kh == 0), stop=(kh == 1))
                        nc.scalar.activation(hw2_sb[:, m, :], psum3[:], AF.Copy)
                    hw2loc_dram = dram_pool.tile([ML, C], bf16)
                    nc.sync.dma_start(
                        hw2loc_dram.rearrange("(mt p) c -> p mt c", p=P),
                        hw2_sb[:])
                    hw2full_dram = dram_pool.tile([NK, C], bf16,
                                                  addr_space="Shared")
                    nc.gpsimd.collective_compute(
                        "AllGather", mybir.AluOpType.bypass, replica_groups=rg,
                        ins=[hw2loc_dram[:].opt()], outs=[hw2full_dram[:].opt()])
                    nc.sync.dma_start(
                        hw2f_sb[:],
                        hw2full_dram.rearrange("(kt p) c -> p kt c", p=P))

            # ====== phase 4: logitsT = hW2_full.T @ adjTn; softmax; E_hat ===
            if PHASES >= 4:
                with tc.tile_pool(name="ph4s", bufs=4) as ph4s, \
                     tc.tile_pool(name="ph4", bufs=1) as ph4, \
                     tc.tile_pool(name="ps4", bufs=1, space="PSUM") as ps4, \
                     tc.tile_pool(name="ps4t", bufs=2, space="PSUM") as ps4t:
                    psum_l = ps4.tile([C, ML], f32, name="psum_l")
                    for k in range(KT):
                        if k < NRESN:
                            src = adjn_res[:, k, :]
                        else:
                            adjn_k2 = ph4s.tile([P, ML], bf16, name="adjn_k2")
                            nc.sync.dma_start(adjn_k2[:],
                                              adjTn[k * P:(k + 1) * P, :])
                            src = adjn_k2[:]
                        for (n0, n1) in CHUNKS:
                            nc.tensor.matmul(
                                psum_l[:, n0:n1],
                                hw2f_sb[:, k, :],
                                src[:, n0:n1],
                                start=(k == 0), stop=(k == KT - 1))
                    lt_sb = ph4.tile([C, ML], f32)
                    nc.scalar.activation(lt_sb[:], psum_l[:], AF.Copy)

                    etb_sb = ph4.tile([C, ML], bf16)
                    for m in range(MT):
                        # logits m-tile, natural layout: [C,P] -> [P,C]
                        tp_ps = ps4t.tile([P, C], f32, name="tp_ps")
                        nc.tensor.transpose(tp_ps[:],
                                            lt_sb[:, m * P:(m + 1) * P],
                                            ident[:C, :C])
                        lg_m = ph4.tile([P, C], f32, name="lg_m", bufs=2)
                        nc.vector.tensor_add(lg_m[:], tp_ps[:], b2r_sb[:])
                        # softmax over free axis
                        mx = ph4.tile([P, 1], f32, name="mx", bufs=2)
                        nc.vector.reduce_max(mx[:], lg_m[:],
                                             axis=mybir.AxisListType.X)
                        negmx = ph4.tile([P, 1], f32, name="negmx", bufs=2)
                        nc.vector.tensor_scalar_mul(negmx[:], mx[:], -1.0)
                        ex = ph4.tile([P, C], f32, name="ex", bufs=2)
                        nc.scalar.activation(ex[:], lg_m[:], AF.Exp,
                                             bias=negmx[:])
                        sm = ph4.tile([P, 1], f32, name="sm", bufs=2)
                        nc.vector.reduce_sum(sm[:], ex[:],
                                             axis=mybir.AxisListType.X)
                        rs = ph4.tile([P, 1], f32, name="rs", bufs=2)
                        nc.vector.reciprocal(rs[:], sm[:])
                        # E_hat = ex * rs - 1/C
                        nc.vector.tensor_scalar(
                            e_sb[:, m * C:(m + 1) * C], ex[:], rs[:], -1.0 / C,
                            op0=mybir.AluOpType.mult, op1=mybir.AluOpType.add)
                        # E_hat.T block
                        tp2_ps = ps4t.tile([C, P], f32, name="tp2_ps")
                        nc.tensor.transpose(tp2_ps[:],
                                            e_sb[:, m * C:(m + 1) * C],
                                            ident[:])
                        nc.scalar.activation(et_sb[:, m * P:(m + 1) * P],
                                             tp2_ps[:], AF.Copy)
                        nc.scalar.activation(etb_sb[:, m * P:(m + 1) * P],
                                             tp2_ps[:], AF.Copy)

                    # all-gather E_hat.T blocks -> btcat
                    et_dram = dram_pool.tile([C, ML], bf16)
                    nc.sync.dma_start(et_dram[:], etb_sb[:])
                    btfull0 = dram_pool.tile([P, ML], bf16, addr_space="Shared")
                    nc.gpsimd.collective_compute(
                        "AllGather", mybir.AluOpType.bypass, replica_groups=rg,
                        ins=[et_dram[:].opt()], outs=[btfull0[:].opt()])
                    for r in range(RANKS):
                        nc.sync.dma_start(btcat_sb[:, r * ML:(r + 1) * ML],
                                          btfull0[C * r:C * (r + 1), :])

            if PHASES >= 2:
                cachen.__exit__(None, None, None)

            # ================= phase 5: post-process iterations =============
            if PHASES >= 5:
                with tc.tile_pool(name="res", bufs=1) as res_pool, \
                     tc.tile_pool(name="ph5s", bufs=4) as ph5s, \
                     tc.tile_pool(name="ph5", bufs=1) as ph5, \
                     tc.tile_pool(name="ps5y", bufs=4, space="PSUM") as ps5y, \
                     tc.tile_pool(name="ps5b", bufs=1, space="PSUM") as ps5b:
                    adjr_res = res_pool.tile([P, NRES, ML], bf16)
                    for it in range(NPOST):
                        # Y = B @ H in K-major layout, from gathered B.T blocks
                        for m in range(KT):
                            psum_y = ps5y.tile([P, C], f32, name="psum_y")
                            nc.tensor.matmul(psum_y[:],
                                             btcat_sb[:, m * P:(m + 1) * P],
                                             h_sb[:], start=True, stop=True)
                            nc.scalar.activation(y_sb[:, m, :], psum_y[:],
                                                 AF.Copy)
                        # T.T = Y.T @ adjTr  (accumulate over k-tiles)
                        psum_b = ps5b.tile([C, ML], f32, name="psum_b")
                        for k in range(KT):
                            if k < NRES:
                                if it == 0:
                                    nc.sync.dma_start(
                                        adjr_res[:, k, :],
                                        adjTr[k * P:(k + 1) * P, :])
                                src = adjr_res[:, k, :]
                            else:
                                adjr_k = ph5s.tile([P, ML], bf16, name="adjr_k")
                                nc.sync.dma_start(adjr_k[:],
                                                  adjTr[k * P:(k + 1) * P, :])
                                src = adjr_k[:]
                            for (n0, n1) in CHUNKS:
                                nc.tensor.matmul(
                                    psum_b[:, n0:n1],
                                    y_sb[:, k, :],
                                    src[:, n0:n1],
                                    start=(k == 0), stop=(k == KT - 1))
                        if it < NPOST - 1:
                            btnb = ph5.tile([C, ML], bf16, name="btnb", bufs=2)
                            nc.vector.tensor_add(btnb[:], psum_b[:], et_sb[:])
                            bt_dram = dram_pool.tile([C, ML], bf16,
                                                     name=f"bt_dram{it}")
                            nc.sync.dma_start(bt_dram[:], btnb[:])
                            btfull = dram_pool.tile([P, ML], bf16,
                                                    name=f"btfull{it}",
                                                    addr_space="Shared")
                            nc.gpsimd.collective_compute(
                                "AllGather", mybir.AluOpType.bypass,
                                replica_groups=rg,
                                ins=[bt_dram[:].opt()], outs=[btfull[:].opt()])
                            for r in range(RANKS):
                                nc.sync.dma_start(
                                    btcat_sb[:, r * ML:(r + 1) * ML],
                                    btfull[C * r:C * (r + 1), :])
                        else:
                            btn = ph5.tile([C, ML], f32, name="btn")
                            nc.vector.tensor_add(btn[:], psum_b[:], et_sb[:])
                            outT_sb = ph5.tile([C, ML], f32, name="outT_sb")
                            nc.vector.tensor_scalar_add(outT_sb[:], btn[:],
                                                        1.0 / C)
                            nc.sync.dma_start(outT[:], outT_sb[:])
            else:
                # truncated build: still write the output tensor
                with tc.tile_pool(name="dummy", bufs=1) as dummy:
                    dpad = dummy.tile([C, ML], f32)
                    nc.gpsimd.memset(dpad[:], 0.0)
                    nc.sync.dma_start(outT[:], dpad[:])

    nc.compile()
    return nc


def _get_compiled():
    if "nc" not in _CACHE:
        _CACHE["nc"] = _build_and_compile()
    return _CACHE["nc"]


def _prep_inputs(raw_adj, normed_adj, features, W1, b1, W2, b2, H):
    bf = ml_dtypes.bfloat16
    xTp = np.zeros((F, NK), dtype=bf)
    xTp[:, :NREAL] = np.ascontiguousarray(features.T).astype(bf)
    w1b = np.ascontiguousarray(W1).astype(bf)
    w2b = np.ascontiguousarray(W2).astype(bf)
    hb = np.ascontiguousarray(H).astype(bf)
    b1c = np.asarray(b1, dtype=np.float32).reshape(HID, 1).copy()
    b2rep = np.broadcast_to(np.asarray(b2, dtype=np.float32), (P, C)).copy()
    in_maps = []
    for r in range(RANKS):
        r0 = r * ML
        r1 = min(r0 + ML, NREAL)
        nr = r1 - r0
        an = np.zeros((NK, ML), dtype=bf)
        an[:NREAL, :nr] = np.ascontiguousarray(normed_adj[r0:r1].T).astype(bf)
        ar = np.zeros((NK, ML), dtype=bf)
        ar[:NREAL, :nr] = np.ascontiguousarray(raw_adj[r0:r1].T).astype(bf)
        in_maps.append({
            "adjTn": an, "adjTr": ar, "xT": xTp, "w1": w1b, "w2": w2b,
            "hm": hb, "b1": b1c, "b2r": b2rep,
        })
    return in_maps


def run_on_device(in_maps, trace=False):
    from concourse import bass_utils
    nc = _get_compiled()
    return bass_utils.run_bass_kernel_spmd(
        nc, in_maps, core_ids=list(range(RANKS)), trace=trace)


def kernel(raw_adj, normed_adj, features, y_onehot, train_mask,
           W1, b1, W2, b2, H):
    in_maps = _prep_inputs(np.asarray(raw_adj), np.asarray(normed_adj),
                           np.asarray(features), np.asarray(W1),
                           np.asarray(b1), np.asarray(W2), np.asarray(b2),
                           np.asarray(H))
    res = run_on_device(in_maps)
    parts = []
    for r in range(RANKS):
        o = np.asarray(res.results[r]["outT"], dtype=np.float32)  # [C, ML]
        parts.append(o.T)
    full = np.concatenate(parts, axis=0)[:NREAL]
    return np.ascontiguousarray(full).astype(np.float32)
